# revision 15
# baseline (speedup 1.0000x reference)
"""Trainium2 Bass kernel for nn_DetectorLoss (SIoU detector loss).

Strategy: data-parallel over batch N=16 -> 8 cores x 2 batches.
Per core: dma_gather pulls the per-candidate pred values (256B rows),
one-hot extraction, SIoU/cls math on DVE+ACT (Exp/Ln table set only),
dense obj baseline = 0.375*sum(pobj^2).  A host roundtrip between two
NEFFs carries the single global scalar (iou_mean); phase B applies the
f-mask, dedups duplicate-cell winners via a column-shift trick, and
reduces the loss partials.  Host combines per-core partial sums.
"""

import numpy as np

import concourse.bass as bass
import concourse.mybir as mybir
from concourse import library_config
from concourse.bass import AP
from concourse.library_overlay import lower_extended_insts
from concourse.tile import TileContext
from concourse.bass_utils import run_bass_kernel_spmd

# ---------------- problem constants (hardcoded per spec) ----------------
N, C, H, W = 16, 85, 160, 160
HW = H * W                  # 25600
CHW = C * HW                # 2176000
NCORES = 8
BPC = 2                     # batches per core
SHARD = BPC * CHW           # elems per core shard
M = 4096
NCAND = 4 * M

f32 = mybir.dt.float32
i16 = mybir.dt.int16
Alu = mybir.AluOpType
Act = mybir.ActivationFunctionType
X = mybir.AxisListType.X

MAX_WAITS = 1


def _split_excess_waits(nc):
    """This neuronxcc build rejects TPB_CTRL-class instructions (Drain/NoOp)
    with >1 sem wait; hoist extras onto same-engine Drains placed
    immediately before (Drains are never elided by codegen).  Compute/DMA
    instructions keep their multi-wait encoding."""
    ctrl = (mybir.InstDrain, mybir.InstNoOp, mybir.InstISA)
    for f in nc.m.functions:
        for bb in f.blocks:
            new_list = []
            for ins in bb.instructions:
                si = ins.sync_info
                if si is not None and len(si.on_wait) > MAX_WAITS:
                    waits = list(si.on_wait)
                    excess, keep = waits[:-MAX_WAITS], waits[-MAX_WAITS:]
                    while excess:
                        chunk, excess = excess[:MAX_WAITS], excess[MAX_WAITS:]
                        carrier = mybir.InstDrain(
                            name=nc.get_next_instruction_name(),
                            engine=ins.engine, ins=[], outs=[],
                            bass_is_fusable=False,
                            sync_info=mybir.SyncInfo(on_wait=chunk, on_update=[]),
                        )
                        nc.register_instruction(carrier)
                        new_list.append(carrier)
                    si.on_wait = keep
                new_list.append(ins)
            bb.instructions[:] = new_list


def _bc(ap, reps, dim):
    """Insert a stride-0 broadcast dim of size `reps` at free-dim index
    `dim` (0 = right after partition dim)."""
    pattern = list(ap.ap)
    pattern.insert(dim + 1, [0, reps])
    return AP(tensor=ap.tensor, offset=ap.offset, ap=pattern)


# ---------------- host preparation ----------------

def _prep(preds, targets):
    preds = np.asarray(preds, np.float32)
    targets = np.asarray(targets, np.float32)
    assert preds.shape == (N, C, H, W) and targets.shape[1] == 6

    dt = np.float32
    # build_target, mirroring reference.py exactly (f32/int ops are exact)
    scale = np.array([1, 1, W, H, W, H], dt)
    gt = (targets * scale).astype(dt)
    gt4 = np.broadcast_to(gt, (4, targets.shape[0], 6))
    quad = np.array([[0, 0], [1, 0], [0, 1], [1, 1]], np.int32)
    gij = gt4[..., 2:4].astype(np.int32) + quad[:, None, :]
    m = (np.min(np.where(gij < H, gij, 0), axis=-1) > 0).reshape(-1)
    gi = np.where(m, gij[..., 0].reshape(-1), 0)
    gj = np.where(m, gij[..., 1].reshape(-1), 0)
    b = np.tile(targets[:, 0].astype(np.int32), 4)
    gbox = gt4[..., 2:].reshape(-1, 4).astype(dt)
    gcls = np.tile(targets[:, 1].astype(np.int32), 4)
    cnt_m = max(int(m.sum()), 1)

    # box2-derived constants (f32, same rounding as reference)
    gx, gy, gw, gh = gbox[:, 0], gbox[:, 1], gbox[:, 2], gbox[:, 3]
    half = dt(0.5)
    b2x1 = (gx - gw * half).astype(dt)
    b2x2 = (gx + gw * half).astype(dt)
    b2y1 = (gy - gh * half).astype(dt)
    b2y2 = (gy + gh * half).astype(dt)
    w2 = (b2x2 - b2x1).astype(dt)
    h2 = ((b2y2 - b2y1) + dt(1e-7)).astype(dt)
    area2h = (w2 * h2).astype(dt)
    sx2 = (b2x1 + b2x2).astype(dt)
    sy2 = (b2y1 + b2y2).astype(dt)

    core = b >> 1
    bl = b & 1
    rr = gj * W + gi           # flat cell within a batch image
    row64 = rr >> 6
    phase = rr & 63

    # ---- per (core, batch) packing: dup-cell groups -> same row, adjacent cols
    placements = {}            # (k, lb) -> list of rows, each row = list of cand idx
    max_cols = 0
    for k in range(NCORES):
        for lb in range(BPC):
            cand = np.where((core == k) & (bl == lb))[0]   # ascending orig order
            groups = {}
            for ci in cand:
                if m[ci]:
                    groups.setdefault(rr[ci], []).append(ci)
            grouped = [v for v in groups.values() if len(v) > 1]
            in_group = set(x for v in grouped for x in v)
            singles = [ci for ci in cand if ci not in in_group]
            rows = [[] for _ in range(128)]
            order = sorted(range(128), key=lambda p: p)
            for gmem in sorted(grouped, key=len, reverse=True):
                p = min(range(128), key=lambda q: len(rows[q]))
                rows[p].extend(gmem)
            for ci in singles:
                p = min(range(128), key=lambda q: len(rows[q]))
                rows[p].append(ci)
            placements[(k, lb)] = rows
            max_cols = max(max_cols, max(len(r) for r in rows))

    GB = max_cols              # cols per batch block
    G = BPC * GB
    PB = 128 * GB
    KA, KB = 5 * PB // 16, PB // 16
    KTOT = BPC * (KA + KB)

    NF = 14                    # hostf fields
    per_core = []
    for k in range(NCORES):
        slot = -np.ones((128, G), np.int64)    # candidate index per slot
        grp = np.zeros((128, G), np.int64)     # group id (cell) for sibling masks
        for lb in range(BPC):
            rows = placements[(k, lb)]
            for p in range(128):
                for j, ci in enumerate(rows[p]):
                    slot[p, lb * GB + j] = ci
                    grp[p, lb * GB + j] = ((lb + 1) * (1 << 20) + int(rr[ci])) if m[ci] else 0

        filled = slot >= 0
        sidx = np.where(filled, slot, 0)

        hostf = np.zeros((128, NF, G), np.float32)
        hostf[:, 0] = np.where(filled, gi[sidx], 0).astype(dt)
        hostf[:, 1] = np.where(filled, gj[sidx], 0).astype(dt)
        hostf[:, 2] = np.where(filled, m[sidx], False).astype(dt)
        hostf[:, 3] = (filled & (np.arange(G)[None, :] < GB)).astype(dt)
        hostf[:, 4] = (filled & (np.arange(G)[None, :] >= GB)).astype(dt)
        hostf[:, 5] = np.where(filled, b2x1[sidx], 0.0)
        hostf[:, 6] = np.where(filled, b2y1[sidx], 0.0)
        hostf[:, 7] = np.where(filled, b2x2[sidx], 1.0)
        hostf[:, 8] = np.where(filled, b2y2[sidx], 1.0)
        hostf[:, 9] = np.where(filled, sx2[sidx], 1.0)
        hostf[:, 10] = np.where(filled, sy2[sidx], 1.0)
        hostf[:, 11] = np.where(filled, w2[sidx], 1.0)
        hostf[:, 12] = np.where(filled, h2[sidx], 1.0)
        hostf[:, 13] = np.where(filled, area2h[sidx], 1.0)

        # sibling masks: e1 -> next col is same dup-group, e2 -> col+2 is
        ggrp = grp * (grp > 0)
        e1 = np.zeros((128, G), np.float32)
        e2 = np.zeros((128, G), np.float32)
        e1[:, :-1] = ((ggrp[:, :-1] == ggrp[:, 1:]) & (ggrp[:, :-1] > 0)).astype(dt)
        e2[:, :-2] = ((ggrp[:, :-2] == ggrp[:, 2:]) & (ggrp[:, :-2] > 0)).astype(dt)

        # one-hot for extraction (zero for pad slots -> extracted value 0)
        oneh = np.zeros((128, G, 64), np.float32)
        pp, cc = np.where(filled)
        oneh[pp, cc, phase[slot[pp, cc]]] = 1.0

        # int16 gather row indices
        def wrap16(idxs):
            n = idxs.shape[0]
            base16 = idxs.reshape(n // 16, 16).T.astype(np.int16)   # [16, n/16]
            return np.tile(base16, (8, 1))                          # [128, n/16]

        idx16 = np.zeros((128, KTOT), np.int16)
        off = 0
        for lb in range(BPC):
            blk = slice(lb * GB, (lb + 1) * GB)
            r64 = np.where(filled[:, blk], row64[sidx[:, blk]], 0)  # [128, GB]
            # gather A: channels 0..4, idx j = ch*PB + cb*128 + p
            ja = np.empty((5, GB, 128), np.int64)
            for ch in range(5):
                ja[ch] = (ch * 400 + r64).T                          # [GB, 128]
            idx16[:, off:off + KA] = wrap16(ja.reshape(-1))
            off += KA
            # gather B: class channel, row = gcls*400 + r64
            cls_row = np.where(filled[:, blk], gcls[sidx[:, blk]] * 400
                               + row64[sidx[:, blk]], 0)
            idx16[:, off:off + KB] = wrap16(cls_row.T.reshape(-1))
            off += KB

        shard = np.ascontiguousarray(preds[BPC * k:BPC * (k + 1)]).reshape(-1)
        pobjd = np.ascontiguousarray(
            preds[BPC * k:BPC * (k + 1), 0]).reshape(128, 400)

        per_core.append(dict(
            shard=shard, pobjd=pobjd, idx16=idx16,
            hostf=hostf.reshape(128, NF * G), oneh=oneh.reshape(128, G * 64),
            hostf2=np.concatenate(
                [hostf[:, 2], hostf[:, 3], hostf[:, 4], e1, e2],
                axis=1).astype(np.float32),
        ))

    meta = dict(GB=GB, G=G, PB=PB, KA=KA, KB=KB, KTOT=KTOT, NF=NF,
                cnt_m=cnt_m)
    return per_core, meta


# ---------------- phase A program ----------------

def _build_phase_a(meta):
    GB, G, PB = meta["GB"], meta["G"], meta["PB"]
    KA, KB, KTOT, NF = meta["KA"], meta["KB"], meta["KTOT"], meta["NF"]
    AOUT = 3 * G + 4

    nc = bass.Bass("TRN2", debug=False)
    shard = nc.dram_tensor("shard", [SHARD], f32, kind="ExternalInput")
    idx16 = nc.dram_tensor("idx16", [128, KTOT], i16, kind="ExternalInput")
    hostf = nc.dram_tensor("hostf", [128, NF * G], f32, kind="ExternalInput")
    oneh = nc.dram_tensor("oneh", [128, G * 64], f32, kind="ExternalInput")
    pobjd = nc.dram_tensor("pobjd", [128, 400], f32, kind="ExternalInput")
    aout = nc.dram_tensor("aout", [128, AOUT], f32, kind="ExternalOutput")

    with TileContext(nc) as tc:
        with tc.tile_pool(name="sbuf", bufs=1) as pool:
            nc.gpsimd.load_library(library_config.mlp)

            idx_t = pool.tile([128, KTOT], i16)
            nc.sync.dma_start(out=idx_t[:], in_=idx16.ap())
            hf = pool.tile([128, NF, G], f32)
            nc.sync.dma_start(
                out=hf[:], in_=hostf.ap().rearrange("p (f g) -> p f g", f=NF))
            oh = pool.tile([128, G, 64], f32)
            nc.sync.dma_start(
                out=oh[:], in_=oneh.ap().rearrange("p (g e) -> p g e", e=64))
            pod = pool.tile([128, 400], f32)
            nc.sync.dma_start(out=pod[:], in_=pobjd.ap())

            out_t = pool.tile([128, AOUT], f32)
            nc.vector.memset(out_t[:], 0.0)

            # warm the Exp/Ln ACT table set before the gathers finish
            warm = pool.tile([128, 1], f32)
            nc.vector.memset(warm[:], 1.0)
            nc.scalar.activation(warm[:], warm[:], Act.Exp)
            nc.scalar.activation(warm[:], warm[:], Act.Ln)

            def F(i):            # hostf field view [128, G]
                return hf[:, i, :]

            def F2(i):           # two adjacent fields as [128, 2, G]
                return hf[:, i:i + 2, :]

            # ---- gathers: 6 fields x 256B rows per candidate ----
            gall = []
            sap = shard.ap()
            for lb in range(BPC):
                g6 = pool.tile([128, 6 * GB, 64], f32, name=f"g6_{lb}", tag=f"g6_{lb}")
                base = lb * CHW
                inA = sap[base:base + 5 * HW].rearrange("(r e) -> r e", e=64)
                inB = sap[base + 5 * HW:base + CHW].rearrange(
                    "(r e) -> r e", e=64)
                o = lb * (KA + KB)
                nc.gpsimd.dma_gather(
                    out_ap=g6[:, 0:5 * GB, :], in_ap=inA,
                    idxs_ap=idx_t[:, o:o + KA],
                    num_idxs=5 * PB, num_idxs_reg=5 * PB, elem_size=64,
                    single_packet=False)
                nc.gpsimd.dma_gather(
                    out_ap=g6[:, 5 * GB:6 * GB, :], in_ap=inB,
                    idxs_ap=idx_t[:, o + KA:o + KA + KB],
                    num_idxs=PB, num_idxs_reg=PB, elem_size=64,
                    single_packet=False)
                gall.append(g6)

            # ---- extraction: multiply by one-hot, reduce the 64-lane ----
            ext = pool.tile([128, 6, G], f32)
            for lb in range(BPC):
                prod = pool.tile([128, 6, GB, 64], f32, name=f"prod{lb}", tag=f"prod{lb}")
                oh_b = oh[:, lb * GB:(lb + 1) * GB, :]
                nc.vector.tensor_tensor(
                    out=prod[:],
                    in0=gall[lb][:].rearrange("p (f c) e -> p f c e", f=6),
                    in1=_bc(oh_b, 6, 0),
                    op=Alu.mult)
                nc.vector.tensor_reduce(
                    out=ext[:, :, lb * GB:(lb + 1) * GB], in_=prod[:],
                    axis=X, op=Alu.add)

            epobj = ext[:, 0, :]
            epr01 = ext[:, 1:3, :]
            epr23 = ext[:, 3:5, :]
            ecls = ext[:, 5, :]

            def T2(tag):
                return pool.tile([128, 2, G], f32, name=tag, tag=tag)[:]

            def T1(tag):
                return pool.tile([128, G], f32, name=tag, tag=tag)[:]

            ts = nc.vector.tensor_scalar
            tt = nc.vector.tensor_tensor
            act = nc.scalar.activation

            # tanh(pr01) = 1 - 2/(exp(2x)+1)
            t01 = T2("t01")
            act(t01, epr01, Act.Exp, scale=2.0)
            ts(t01, t01, 1.0, None, Alu.add)
            nc.vector.reciprocal(t01, t01)
            ts(t01, t01, -2.0, 1.0, Alu.mult, Alu.add)
            # pwh2 = 80*sigmoid(pr23) = 80/(1+exp(-x))
            pwh2 = T2("pwh2")
            act(pwh2, epr23, Act.Exp, scale=-1.0)
            ts(pwh2, pwh2, 1.0, None, Alu.add)
            nc.vector.reciprocal(pwh2, pwh2)
            ts(pwh2, pwh2, 80.0, None, Alu.mult)

            gij_f = hf[:, 0:2, :]
            txy = T2("txy")
            tt(txy, t01, gij_f, Alu.add)
            b1a = T2("b1a")
            tt(b1a, txy, pwh2, Alu.subtract)
            b1b = T2("b1b")
            tt(b1b, txy, pwh2, Alu.add)

            wh1 = T2("wh1")
            tt(wh1, b1b, b1a, Alu.subtract)
            ts(wh1[:, 1, :], wh1[:, 1, :], 1e-7, None, Alu.add)  # h1 += eps

            area1 = T1("area1")
            tt(area1, wh1[:, 0, :], wh1[:, 1, :], Alu.mult)

            b2a = F2(5)       # (b2x1, b2y1)
            b2b = F2(7)       # (b2x2, b2y2)
            mn = T2("mn")
            tt(mn, b1b, b2b, Alu.min)
            mx = T2("mx")
            tt(mx, b1a, b2a, Alu.max)
            dcl = T2("dcl")
            tt(dcl, mn, mx, Alu.subtract)
            ts(dcl, dcl, 0.0, None, Alu.max)
            inter = T1("inter")
            tt(inter, dcl[:, 0, :], dcl[:, 1, :], Alu.mult)

            u = T1("u")
            tt(u, area1, F(13), Alu.add)
            tt(u, u, inter, Alu.subtract)
            ts(u, u, 1e-7, None, Alu.add)
            invu = T1("invu")
            nc.vector.reciprocal(invu, u)
            iou0 = T1("iou0")
            tt(iou0, inter, invu, Alu.mult)

            cwh = T2("cwh")
            mx2 = T2("mx2")
            tt(mx2, b1b, b2b, Alu.max)
            mn2 = T2("mn2")
            tt(mn2, b1a, b2a, Alu.min)
            tt(cwh, mx2, mn2, Alu.subtract)

            scw = T2("scw")
            tt(scw, F2(9), b1a, Alu.subtract)       # (sx2,sy2) - b1x1y1
            tt(scw, scw, b1b, Alu.subtract)
            ts(scw, scw, 0.5, None, Alu.mult)

            sq = T2("sq")
            tt(sq, scw, scw, Alu.mult)
            ssum = T1("ssum")
            tt(ssum, sq[:, 0, :], sq[:, 1, :], Alu.add)
            invsig = T1("invsig")
            act(invsig, ssum, Act.Ln)
            act(invsig, invsig, Act.Exp, scale=-0.5)   # rsqrt via exp/ln

            negs = T2("negs")
            ts(negs, scw, -1.0, None, Alu.mult)
            sabs = T2("sabs")
            tt(sabs, scw, negs, Alu.max)
            sin1 = T1("sin1")
            tt(sin1, sabs[:, 0, :], invsig, Alu.mult)
            sin2 = T1("sin2")
            tt(sin2, sabs[:, 1, :], invsig, Alu.mult)

            thr = float(np.float32(2 ** 0.5 / 2))
            thr_t = pool.tile([128, 1], f32, name="thr_t")
            nc.vector.memset(thr_t[:], thr)
            cgt = T1("cgt")
            tt(cgt, sin1, thr_t[:].to_broadcast([128, G]), Alu.is_gt)
            dsin = T1("dsin")
            tt(dsin, sin2, sin1, Alu.subtract)
            tt(dsin, cgt, dsin, Alu.mult)
            sina = T1("sina")
            tt(sina, sin1, dsin, Alu.add)

            # angle_cost = 2*sina*sqrt(1-sina^2); gamma = angle_cost-2
            sa2 = T1("sa2")
            tt(sa2, sina, sina, Alu.mult)
            om = T1("om")
            ts(om, sa2, -1.0, 1.0, Alu.mult, Alu.add)
            rt = T1("rt")
            act(rt, om, Act.Ln)
            act(rt, rt, Act.Exp, scale=0.5)            # sqrt via exp/ln
            gam = T1("gam")
            tt(gam, sina, rt, Alu.mult)
            ts(gam, gam, 2.0, -2.0, Alu.mult, Alu.add)

            invcw = T2("invcw")
            nc.vector.reciprocal(invcw, cwh)
            rho = T2("rho")
            tt(rho, scw, invcw, Alu.mult)
            tt(rho, rho, rho, Alu.mult)
            gr = T2("gr")
            tt(gr[:, 0, :], gam, rho[:, 0, :], Alu.mult)
            tt(gr[:, 1, :], gam, rho[:, 1, :], Alu.mult)
            act(gr, gr, Act.Exp)
            dist = T1("dist")
            tt(dist, gr[:, 0, :], gr[:, 1, :], Alu.add)
            ts(dist, dist, -1.0, 2.0, Alu.mult, Alu.add)

            wh2t = F2(11)
            dwh = T2("dwh")
            tt(dwh, wh1, wh2t, Alu.subtract)
            ts(negs, dwh, -1.0, None, Alu.mult)
            tt(dwh, dwh, negs, Alu.max)
            mxw = T2("mxw")
            tt(mxw, wh1, wh2t, Alu.max)
            nc.vector.reciprocal(mxw, mxw)
            omg = T2("omg")
            tt(omg, dwh, mxw, Alu.mult)
            act(omg, omg, Act.Exp, scale=-1.0)
            ts(omg, omg, -1.0, 1.0, Alu.mult, Alu.add)   # 1-exp(-omiga)
            tt(omg, omg, omg, Alu.mult)                  # ^2
            tt(omg, omg, omg, Alu.mult)                  # ^4
            shp = T1("shp")
            tt(shp, omg[:, 0, :], omg[:, 1, :], Alu.add)

            dsh = T1("dsh")
            tt(dsh, dist, shp, Alu.add)
            ts(dsh, dsh, -0.5, None, Alu.mult)
            iou_v = out_t[:, 0:G]
            tt(iou_v, iou0, dsh, Alu.add)

            # sum(iou*m) partial per partition
            scr = T1("scr")
            tt(scr, iou_v, F(2), Alu.mult)
            nc.vector.tensor_reduce(out=out_t[:, 3 * G:3 * G + 1], in_=scr,
                                    axis=X, op=Alu.add)

            # lnp
            pg = T1("pg")
            ts(pg, ecls, 1e-38, None, Alu.max)
            act(out_t[:, G:2 * G], pg, Act.Ln)

            # pobj at candidate cells, for phase B
            nc.vector.tensor_copy(out=out_t[:, 2 * G:3 * G], in_=epobj)

            # dense obj baseline partial: sum(pobj^2) per partition
            scr4 = pool.tile([128, 400], f32)
            nc.vector.tensor_tensor(out=scr4[:], in0=pod[:], in1=pod[:],
                                    op=Alu.mult)
            nc.vector.tensor_reduce(out=out_t[:, 3 * G + 1:3 * G + 2],
                                    in_=scr4[:], axis=X, op=Alu.add)

            nc.sync.dma_start(out=aout.ap(), in_=out_t[:])

    lower_extended_insts(nc)
    _split_excess_waits(nc)
    return nc


# ---------------- phase B program ----------------

def _build_phase_b(meta):
    G = meta["G"]
    AOUT = 3 * G + 4

    nc = bass.Bass("TRN2", debug=False)
    bin_ = nc.dram_tensor("bin", [128, AOUT], f32, kind="ExternalInput")
    hostf2 = nc.dram_tensor("hostf2", [128, 5 * G], f32, kind="ExternalInput")
    imean = nc.dram_tensor("imean", [128, 1], f32, kind="ExternalInput")
    bout = nc.dram_tensor("bout", [128, 8], f32, kind="ExternalOutput")

    with TileContext(nc) as tc:
        with tc.tile_pool(name="sbuf", bufs=1) as pool:
            nc.gpsimd.load_library(library_config.mlp)

            bi = pool.tile([128, AOUT], f32)
            nc.sync.dma_start(out=bi[:], in_=bin_.ap())
            h2 = pool.tile([128, 5, G], f32)
            nc.sync.dma_start(
                out=h2[:], in_=hostf2.ap().rearrange("p (f g) -> p f g", f=5))
            im = pool.tile([128, 1], f32)
            nc.sync.dma_start(out=im[:], in_=imean.ap())
            ob = pool.tile([128, 8], f32)
            nc.vector.memset(ob[:], 0.0)

            iou_v = bi[:, 0:G]
            lnp_v = bi[:, G:2 * G]
            pox = bi[:, 2 * G:3 * G]
            m_v, mk0, mk1 = h2[:, 0, :], h2[:, 1, :], h2[:, 2, :]
            e1_v, e2_v = h2[:, 3, :], h2[:, 4, :]

            ts = nc.vector.tensor_scalar
            tt = nc.vector.tensor_tensor

            fpad = pool.tile([128, G + 2], f32)
            nc.vector.memset(fpad[:], 0.0)
            f_v = fpad[:, 0:G]
            tt(f_v, iou_v, im[:, 0:1].to_broadcast([128, G]), Alu.is_gt)
            tt(f_v, f_v, m_v, Alu.mult)

            # winner mask: W = f * (1 - e1*f[:,c+1]) * (1 - e2*f[:,c+2])
            t1 = pool.tile([128, G], f32, name="t1", tag="t1")[:]
            tt(t1, e1_v, fpad[:, 1:G + 1], Alu.mult)
            ts(t1, t1, -1.0, 1.0, Alu.mult, Alu.add)
            t2 = pool.tile([128, G], f32, name="t2", tag="t2")[:]
            tt(t2, e2_v, fpad[:, 2:G + 2], Alu.mult)
            ts(t2, t2, -1.0, 1.0, Alu.mult, Alu.add)
            W_v = pool.tile([128, G], f32, name="W", tag="W")[:]
            tt(W_v, f_v, t1, Alu.mult)
            tt(W_v, W_v, t2, Alu.mult)

            # nperb (both batches) -> all partitions
            fm = pool.tile([128, 2, G], f32)
            tt(fm[:, 0, :], f_v, mk0, Alu.mult)
            tt(fm[:, 1, :], f_v, mk1, Alu.mult)
            np2 = pool.tile([128, 2], f32)
            nc.vector.tensor_reduce(out=np2[:], in_=fm[:], axis=X, op=Alu.add)
            npa = pool.tile([128, 2], f32)
            import concourse.bass_isa as bass_isa
            nc.gpsimd.partition_all_reduce(
                npa[:], np2[:], channels=128,
                reduce_op=bass_isa.ReduceOp.add)
            ts(npa[:], npa[:], 0.5, None, Alu.max)
            nc.vector.tensor_copy(out=ob[:, 4:6], in_=npa[:])
            inv = pool.tile([128, 2], f32)
            nc.vector.reciprocal(inv[:], npa[:])

            fv = pool.tile([128, G], f32, name="fv", tag="fv")[:]
            t3 = pool.tile([128, G], f32, name="t3", tag="t3")[:]
            ts(t3, mk0, inv[:, 0:1], None, Alu.mult)
            ts(fv, mk1, inv[:, 1:2], None, Alu.mult)
            tt(fv, fv, t3, Alu.add)
            ts(fv, fv, 6400.0, None, Alu.mult)

            # obj correction: W*(sl1(pobj-iou)*fval - 0.375*pobj^2)
            d = pool.tile([128, G], f32, name="d", tag="d")[:]
            tt(d, pox, iou_v, Alu.subtract)
            ad = pool.tile([128, G], f32, name="ad", tag="ad")[:]
            ts(ad, d, -1.0, None, Alu.mult)
            tt(ad, d, ad, Alu.max)
            one_t = pool.tile([128, 1], f32, name="one_t")
            nc.vector.memset(one_t[:], 1.0)
            cc = pool.tile([128, G], f32, name="cc", tag="cc")[:]
            tt(cc, ad, one_t[:].to_broadcast([128, G]), Alu.is_lt)
            q = pool.tile([128, G], f32, name="q", tag="q")[:]
            tt(q, d, d, Alu.mult)
            ts(q, q, 0.5, None, Alu.mult)
            l_ = pool.tile([128, G], f32, name="l_", tag="l_")[:]
            ts(l_, ad, 0.5, None, Alu.subtract)
            tt(q, q, l_, Alu.subtract)
            tt(q, cc, q, Alu.mult)
            tt(q, l_, q, Alu.add)          # q = sl1
            tt(q, q, fv, Alu.mult)
            po2 = pool.tile([128, G], f32, name="po2", tag="po2")[:]
            tt(po2, pox, pox, Alu.mult)
            ts(po2, po2, 0.375, None, Alu.mult)
            tt(q, q, po2, Alu.subtract)
            scr = pool.tile([128, G], f32, name="scr", tag="scr")[:]
            tt(scr, W_v, q, Alu.mult)
            nc.vector.tensor_reduce(out=ob[:, 3:4], in_=scr, axis=X, op=Alu.add)

            # S1 = sum f*(1-iou); S2 = sum f*lnp; cntf = sum f
            onem = pool.tile([128, G], f32, name="onem", tag="onem")[:]
            ts(onem, iou_v, -1.0, 1.0, Alu.mult, Alu.add)
            s1t = pool.tile([128, G], f32, name="s1t", tag="s1t")[:]
            tt(s1t, f_v, onem, Alu.mult)
            nc.vector.tensor_reduce(out=ob[:, 0:1], in_=s1t, axis=X, op=Alu.add)
            s2t = pool.tile([128, G], f32, name="s2t", tag="s2t")[:]
            tt(s2t, f_v, lnp_v, Alu.mult)
            nc.vector.tensor_reduce(out=ob[:, 1:2], in_=s2t, axis=X, op=Alu.add)
            nc.vector.tensor_reduce(out=ob[:, 2:3], in_=f_v, axis=X,
                                    op=Alu.add)

            nc.sync.dma_start(out=bout.ap(), in_=ob[:])

    lower_extended_insts(nc)
    _split_excess_waits(nc)
    return nc


# ---------------- main entry ----------------

_CACHE = {}


def kernel(preds, targets):
    per_core, meta = _prep(preds, targets)

    key = (meta["GB"],)
    if key not in _CACHE:
        _CACHE[key] = (_build_phase_a(meta), _build_phase_b(meta))
    nc_a, nc_b = _CACHE[key]

    core_ids = list(range(NCORES))
    in_maps_a = [dict(shard=d["shard"], idx16=d["idx16"], hostf=d["hostf"],
                      oneh=d["oneh"], pobjd=d["pobjd"]) for d in per_core]
    res_a = run_bass_kernel_spmd(nc_a, in_maps_a, core_ids)

    G = meta["G"]
    aouts = [res_a.results[k]["aout"] for k in core_ids]
    sum_im = sum(float(a[:, 3 * G].sum(dtype=np.float64)) for a in aouts)
    base = sum(float(a[:, 3 * G + 1].sum(dtype=np.float64)) for a in aouts)
    iou_mean = np.float32(sum_im) / np.float32(meta["cnt_m"])

    imean_arr = np.full((128, 1), iou_mean, np.float32)
    in_maps_b = [dict(bin=aouts[k], hostf2=per_core[k]["hostf2"],
                      imean=imean_arr) for k in core_ids]
    res_b = run_bass_kernel_spmd(nc_b, in_maps_b, core_ids)

    bouts = [res_b.results[k]["bout"] for k in core_ids]
    S1 = sum(float(o[:, 0].sum(dtype=np.float64)) for o in bouts)
    S2 = sum(float(o[:, 1].sum(dtype=np.float64)) for o in bouts)
    cnt_f = max(sum(float(o[:, 2].sum(dtype=np.float64)) for o in bouts), 1.0)
    corr = sum(float(o[:, 3].sum(dtype=np.float64)) for o in bouts)

    iou_loss = np.float32(S1 / cnt_f)
    cls_loss = np.float32(-S2 / cnt_f)
    obj_loss = np.float32((0.375 * base + corr) / (N * HW))
    loss = np.float32(iou_loss * 8 + obj_loss * 16 + cls_loss)
    return (iou_loss, obj_loss, cls_loss, loss)
